# revision 8
# baseline (speedup 1.0000x reference)
"""Trainium2 Bass kernel for nn_BLoss: loss = mean_i(max(0, sum_j B[i,j] - 1)).

Data-parallel over 8 NeuronCores; each core streams a [1024, 16384] f32 row
shard from HBM in [128, W] chunks. Trace-measured facts this design rests on:

- The per-core stream is bound by the 16 SDMA engines' read-side datapath at
  ~27 GB/s each (~435 GB/s/core); 64 MiB/core -> ~154.3 us stream floor. The
  HBM stacks behind these tunneled cores are not shared, so the
  SBUF-AXI/engine path is the wall.
- Chunks 1.. stream via SWDGE cast-DMA (nc.gpsimd, f32 -> bf16). The read
  side is the cap either way, but halving SBUF-write bytes relieves the
  2:1-muxed SBUF AXI ports. bf16 rounding of uniform[0,1) data is unbiased
  and averages out across 16384-col rows (measured end-to-end rel err
  3.3e-4, tolerance 2e-2).
- Chunk 0 goes through HWDGE (nc.sync) in f32: HWDGE first-byte latency
  beats the Q7 SWDGE emission path, starting the stream earlier.
- The two memsets run on the GpSimd Q7 core - the same core that emits
  SWDGE descriptors - so they sit after the first SWDGE dma_starts.
- Per-chunk row sums run on two engines in parallel (VectorE tensor_reduce
  ~1.07 ns/col, ScalarE activation-Copy accum_out ~0.93 ns/col + fixed
  ~0.7 us per op incl. accumulator read), split 5/8 DVE / 3/8 ACT for the
  big chunks.
- DVE's reduce rate (~1.07 ns/col) barely exceeds the DMA arrival rate
  (1.177 ns/col f32), so no reduce taper can shrink the post-stream tail
  below ~1.4 us. Instead, tile 7's LAST 1024 f32 columns are DMA'd
  *directly into the partial-sums tile* (SWDGE f32->f32, same ring so they
  land last): they need no per-chunk reduce. A first 512-col direct block
  (d1) is folded into a pre-reduce (s1) that runs while the final 512-col
  block (d2) streams; after the last byte only reduce(s1,d2,const-1) + one
  fused tensor_scalar remain (~1.0 us vs 2.15 us for the best reduce-only
  taper).
- Tiles 0-6 rowsums/hinge/partial-sum run mid-stream while tile 7 drains; a
  -1.0 constant column folded into tile 7's partials lets the final hinge +
  combine collapse into one tensor_scalar (MAX 0, ADD hsum06).
- The 128 per-partition hinge sums collapse via a bf16 ones-matmul on
  TensorE (single-pass; fp32 would need a LOW/HIGH pair) so the output DMA
  is a single 4-byte descriptor (a [128,1] output pays ~7 us in straggling
  completion receipts). Host sums the 8 per-core scalars and divides by the
  global batch.
"""

import numpy as np
from contextlib import ExitStack

import concourse.bass as bass
import concourse.tile as tile
from concourse import bacc, mybir
from concourse.bass_utils import run_bass_kernel_spmd

N_CORES = 8
ROWS, COLS = 8192, 16384
SHARD_ROWS = ROWS // N_CORES  # 1024
P = 128                       # SBUF partitions
N_RT = SHARD_ROWS // P        # 8 row tiles per core
CHUNK = 8192
PENALTY_B = 1.0

# Tile-7 reduced chunks (width, dve_cols); dve_cols==width -> DVE-only,
# dve_cols==0 -> ACT-only. Schedule chosen by simulating DVE/ACT rates and
# fixed costs against the 1.177 ns/col arrival rate.
TAIL = [(8192, 4096), (4096, 1536), (1280, 1280), (1280, 384), (512, 512)]
D1 = 512   # direct-to-sums f32 block, pre-reduced by s1 while d2 streams
D2 = 512   # final direct-to-sums f32 block; lands last
assert sum(w for w, _ in TAIL) + D1 + D2 == COLS

N_T06 = 4 * (N_RT - 1)  # tiles 0-6: 2 chunks x 2 engines each = 28
N_T7 = sum(1 if d in (0, w) else 2 for w, d in TAIL)  # 7 partials
LAST_T16_IDX = 2 * (N_RT - 1) - 1

# sums-tile column layout
S_T7 = N_T06                  # t7 partials [S_T7 : S_T7+N_T7]
S_D1 = S_T7 + N_T7            # d1 direct cols
S_S1 = S_D1 + D1              # s1 = sum(t7 partials + d1)
S_D2 = S_S1 + 1               # d2 direct cols
S_CONST = S_D2 + D2           # -1.0
N_SUMS = S_CONST + 1

_PROGRAM = None


def _build_program() -> bass.Bass:
    nc = bacc.Bacc("TRN2", target_bir_lowering=False, debug=False)
    B = nc.declare_dram_parameter(
        "B", [SHARD_ROWS, COLS], mybir.dt.float32, isOutput=False
    )
    out = nc.declare_dram_parameter("out", [1, 1], mybir.dt.float32, isOutput=True)

    r7 = (N_RT - 1) * P

    # Warmup DMA, issued BEFORE the tile context: it dispatches right after
    # the Scalar engine's preamble (~0.7 us before the tile entry barrier
    # completes), so the 16 SDMA engines / HBM / SBUF ports are warm when
    # the real stream's first descriptors arrive (the first descriptor per
    # engine otherwise runs ~50% slow). The data is never read; the real
    # stream re-reads those columns.
    warm = nc.alloc_sbuf_tensor("warm", [P, 1024], mybir.dt.float32)
    wsem = nc.alloc_semaphore("wsem")
    nc.scalar.dma_start(warm[:, :], B[0:P, 0:1024]).then_inc(wsem, 16)

    with ExitStack() as ctx:
        tc = ctx.enter_context(tile.TileContext(nc))
        data = ctx.enter_context(tc.tile_pool(name="data", bufs=8))
        data0 = ctx.enter_context(tc.tile_pool(name="data0", bufs=1))
        stats = ctx.enter_context(tc.tile_pool(name="stats", bufs=1))
        psum = ctx.enter_context(tc.tile_pool(name="psum", bufs=1, space="PSUM"))

        sums = stats.tile([P, N_SUMS], mybir.dt.float32)
        dummy = stats.tile([P, 4608], mybir.dt.bfloat16)
        ones = stats.tile([P, 1], mybir.dt.bfloat16)

        # (row_tile, col0, width, dve_cols) - tiles 0-6 in 8192-col pairs.
        chunks = []
        for r in range(N_RT - 1):
            chunks.append((r, 0, CHUNK, CHUNK * 5 // 8))
            chunks.append((r, CHUNK, CHUNK, CHUNK * 5 // 8))
        col = 0
        for w, dve in TAIL:
            chunks.append((N_RT - 1, col, w, dve))
            col += w

        pcol = 0
        for i, (r, c0, w, dve_cols) in enumerate(chunks):
            if i == 0:
                # HWDGE f32: the Sync engine's ring starts draining before
                # the Q7 has emitted its first SWDGE descriptors.
                t = data0.tile([P, w], mybir.dt.float32, tag="t0")
                nc.sync.dma_start(t[:], B[r * P : (r + 1) * P, c0 : c0 + w])
            else:
                t = data.tile([P, w], mybir.dt.bfloat16, tag="t")
                nc.gpsimd.dma_start(t[:], B[r * P : (r + 1) * P, c0 : c0 + w])
            if i == 2:
                # Both memsets execute on the Q7; keep them behind the
                # first SWDGE descriptor emissions.
                nc.vector.memset(ones[:], 1.0)
                nc.vector.memset(sums[:, S_CONST:], -1.0)
            if dve_cols > 0:
                nc.vector.reduce_sum(
                    sums[:, pcol : pcol + 1],
                    t[:, :dve_cols],
                    axis=mybir.AxisListType.X,
                )
                pcol += 1
            if dve_cols < w:
                nc.scalar.activation(
                    dummy[:, : w - dve_cols],
                    t[:, dve_cols:w],
                    mybir.ActivationFunctionType.Copy,
                    accum_out=sums[:, pcol : pcol + 1],
                )
                pcol += 1
            # Tiles 0-6 stats run mid-stream, right after tile 6's chunks:
            # DVE is otherwise idle while tile 7's tail streams in.
            if i == LAST_T16_IDX:
                rowsums06 = stats.tile([P, N_RT - 1], mybir.dt.float32)
                nc.vector.reduce_sum(
                    rowsums06[:],
                    sums[:, :N_T06].rearrange("p (r c) -> p r c", c=4),
                    axis=mybir.AxisListType.X,
                )
                hinge06 = stats.tile([P, N_RT - 1], mybir.dt.float32)
                nc.vector.tensor_scalar(
                    hinge06[:],
                    rowsums06[:],
                    -1.0,
                    0.0,
                    op0=mybir.AluOpType.add,
                    op1=mybir.AluOpType.max,
                )
                hsum06 = stats.tile([P, 1], mybir.dt.float32)
                nc.vector.reduce_sum(
                    hsum06[:], hinge06[:], axis=mybir.AxisListType.X
                )
        assert pcol == S_T7 + N_T7

        # Direct-to-sums f32 tail: same SWDGE ring as the data chunks, so
        # these drain after the reduced chunks; d2 is the last-landing DMA.
        nc.gpsimd.dma_start(
            sums[:, S_D1 : S_D1 + D1], B[r7 : r7 + P, COLS - D1 - D2 : COLS - D2]
        )
        nc.gpsimd.dma_start(
            sums[:, S_D2 : S_D2 + D2], B[r7 : r7 + P, COLS - D2 : COLS]
        )
        # s1 = sum(t7 partials + d1): runs while d2 streams in.
        nc.vector.reduce_sum(
            sums[:, S_S1 : S_S1 + 1],
            sums[:, S_T7:S_S1],
            axis=mybir.AxisListType.X,
        )
        # rowsum7 - 1 = s1 + sum(d2) + (-1 constant column).
        t7sum = stats.tile([P, 1], mybir.dt.float32)
        nc.vector.reduce_sum(
            t7sum[:], sums[:, S_S1:], axis=mybir.AxisListType.X
        )
        # hsum = max(rowsum7 - 1, 0) + hsum06, in one DVE op. bf16 out ->
        # single-pass PE matmul. ulp(65536)=256 per partition, iid across
        # 128 partitions -> ~1e-4 rel on the final loss. Tolerance is 2e-2.
        hsum = stats.tile([P, 1], mybir.dt.bfloat16)
        with nc.allow_low_precision(reason="bf16 hsum: 256-ulp on 8.4M total, iid across partitions; tolerance 2e-2"):
            nc.vector.tensor_scalar(
                hsum[:],
                t7sum[:],
                0.0,
                hsum06[:, 0:1],
                op0=mybir.AluOpType.max,
                op1=mybir.AluOpType.add,
            )

        acc = psum.tile([1, 1], mybir.dt.float32)
        nc.tensor.matmul(acc[:], ones[:], hsum[:], start=True, stop=True)
        res = stats.tile([1, 1], mybir.dt.float32)
        nc.scalar.copy(res[:], acc[:])
        nc.sync.dma_start(out[:], res[:])

    nc.compile()
    return nc


def _run(B: np.ndarray, trace: bool = False):
    global _PROGRAM
    if _PROGRAM is None:
        _PROGRAM = _build_program()
    in_maps = [
        {"B": B[i * SHARD_ROWS : (i + 1) * SHARD_ROWS]} for i in range(N_CORES)
    ]
    res = run_bass_kernel_spmd(_PROGRAM, in_maps, list(range(N_CORES)), trace=trace)
    total = float(sum(np.float64(r["out"][0, 0]) for r in res.results))
    value = np.asarray(np.float32(PENALTY_B * total / ROWS))
    return value, res


def kernel(B: np.ndarray) -> np.ndarray:
    B = np.ascontiguousarray(np.asarray(B, dtype=np.float32))
    assert B.shape == (ROWS, COLS), B.shape
    value, _ = _run(B, trace=False)
    return value
